# revision 27
# baseline (speedup 1.0000x reference)
"""Trainium2 Bass kernel for histogram_binning (spatial hash + Gaussian splat).

kernel(points, mask, atom_types) -> (dense_lattice, buffer_, buffer_mask)

Spatial sharding: core k owns x-voxel slab [16k,16k+16) / x-bins [8k,8k+8).
Host shards points by x (with 1-voxel halo for the splat) into SoA tile-major
arrays; a single SPMD program runs on all 8 cores (the per-core x offset is
folded into the coordinates host-side: x' = x - 4k + 1). On device:
  * batched vector math derives voxel/bin ids and Gaussian x-weights
  * a per-tile counting unit (PE broadcast-transpose + equality one-hots +
    matmul gather/update of a running [128,256] bin-count table) produces each
    point's exact stable slot within its spatial bin
  * winning (x,y,z) records are scattered into buffer_ by indirect DMA;
    buffer_mask comes from the final count table
  * the lattice is built by separable-Gaussian matmuls: for each (x-plane,
    type) group, PSUM accumulates AY^T @ (wx_o * AZ) [y,z] planes, folded into
    a rolling 3-plane SBUF slab and DMA'd out per completed x-plane.
Outputs are disjoint across cores; the host concatenates. No collectives.
"""
import numpy as np

BOX = 32.0
D = 64
L = 128
T = 8
GW = 0.3
CAP = 8
NCORES = 8
XB_PER = D // NCORES          # 8 x-bins per core
XV_PER = L // NCORES          # 16 x-voxels per core
NBL = XB_PER * D * D          # 32768 local bins per core
NGRP = (XV_PER + 2) * T       # 144 (x-plane, type) splat groups
AEXP = (BOX / L) ** 2 / (2.0 * GW * GW)
PADV = 1.0e6
SENT = 40000.0

TB = 1120                     # buffer tiles (of 128 points)
GTILES = 10                   # tiles per splat group

_PROG_CACHE = {}


# ----------------------------------------------------------------------
def _host_shard(points, mask, atom_types, tb, gtiles):
    pts = np.ascontiguousarray(np.asarray(points, np.float32))
    msk = np.asarray(mask).astype(bool)
    att = np.asarray(atom_types).astype(np.int64)
    x = pts[:, 0]
    xb = np.clip(np.floor(x * (D / BOX)).astype(np.int32), 0, D - 1)
    cx = np.clip(np.floor(x * (L / BOX)).astype(np.int32), 0, L - 1)
    yb = np.clip(np.floor(pts[:, 1] * (D / BOX)).astype(np.int32), 0, D - 1)
    zb = np.clip(np.floor(pts[:, 2] * (D / BOX)).astype(np.int32), 0, D - 1)
    cyg = np.floor(pts[:, 1] * (L / BOX)).astype(np.float32)
    czg = np.floor(pts[:, 2] * (L / BOX)).astype(np.float32)
    d0g = (np.floor(pts[:, 0] * (L / BOX))
           - (pts[:, 0].astype(np.float64) * (L / BOX) - 0.5)).astype(np.float32)
    gcap = gtiles * 128
    parts = np.arange(128, dtype=np.float32)
    cores = []
    for k in range(NCORES):
        ib = np.nonzero((xb >= k * XB_PER) & (xb < (k + 1) * XB_PER) & msk)[0]
        nb_cap = tb * 128
        if len(ib) > nb_cap:
            raise RuntimeError(f"core {k}: {len(ib)} buffer points > cap {nb_cap}")
        nb = len(ib)
        bx = np.full(nb_cap, PADV, np.float32)
        by = np.full(nb_cap, PADV, np.float32)
        bz = np.full(nb_cap, PADV, np.float32)
        bv = np.zeros(nb_cap, np.float32)
        binv = np.empty(nb_cap, np.float32)
        binv.reshape(-1, 128)[:] = SENT + parts[None, :]   # sentinel + partition
        bx[:nb] = pts[ib, 0]
        by[:nb] = pts[ib, 1]
        bz[:nb] = pts[ib, 2]
        bv[:nb] = 1.0
        binv[:nb] = (((xb[ib] - k * XB_PER) * D + yb[ib]) * D
                     + zb[ib]).astype(np.float32)
        hivv = np.floor(binv / 256.0).astype(np.float32)
        lovv = binv - hivv * 256.0
        isp = np.nonzero((cx >= k * XV_PER - 1) & (cx <= (k + 1) * XV_PER) & msk)[0]
        g = (cx[isp] - k * XV_PER + 1) * T + att[isp]
        order = np.argsort(g, kind="stable")
        isp = isp[order]
        g = g[order]
        sy = np.full(NGRP * gcap, PADV, np.float32)
        sz = np.full(NGRP * gcap, PADV, np.float32)
        sv = np.zeros(NGRP * gcap, np.float32)
        cyc = np.full(NGRP * gcap, PADV, np.float32)
        czc = np.full(NGRP * gcap, PADV, np.float32)
        d0c = np.zeros(NGRP * gcap, np.float32)
        starts = np.searchsorted(g, np.arange(NGRP + 1))
        for gi in range(NGRP):
            rows = isp[starts[gi]:starts[gi + 1]]
            cnt = len(rows)
            if cnt > gcap:
                raise RuntimeError(f"core {k}: group {gi} count {cnt} > cap {gcap}")
            o = gi * gcap
            sy[o:o + cnt] = pts[rows, 1]
            sz[o:o + cnt] = pts[rows, 2]
            sv[o:o + cnt] = 1.0
            cyc[o:o + cnt] = cyg[rows]
            czc[o:o + cnt] = czg[rows]
            d0c[o:o + cnt] = d0g[rows]

        def tm(a):
            return np.ascontiguousarray(a.reshape(-1, 128).T)
        cores.append({"bxa": tm(bx), "bya": tm(by), "bza": tm(bz), "vba": tm(bv),
                      "binv": tm(binv), "hiv": tm(hivv), "lov": tm(lovv),
                      "sya": tm(sy), "sza": tm(sz), "vla": tm(sv),
                      "cyv": tm(cyc), "czv": tm(czc), "d0x": tm(d0c)})
    return cores


# ----------------------------------------------------------------------
def _patch_tile_drain():
    """This walrus build allows one sem-wait per instruction; Tile's final
    drain carries several — split them across multiple drains."""
    import concourse.tile as tile_mod
    from concourse.vector_clock import ScopedClock
    import bass_rust

    def _split(self, tick_clock, wait_clock):
        nc = self.nc
        drain_inst = nc.sync.drain()
        wait_clock.add_sem_waits(
            drain_inst.ins, ScopedClock({None: tick_clock.global_clock}))
        si = drain_inst.ins.sync_info
        if si is not None and si.on_wait and len(si.on_wait) > 1:
            waits = list(si.on_wait)
            si.on_wait = waits[:1]
            for w in waits[1:]:
                d2 = nc.sync.drain()
                d2.ins.sync_info = bass_rust.SyncInfo(on_wait=[w], on_update=[])
        nc.all_engine_barrier()
        assert self.sems is not None
        popped = nc._tile_sem_poison_stack.pop()
        assert popped is self._sem_poison
        nc.clear_and_free_semaphores(list(self.sems.allocated().values()))
        nc.all_engine_barrier()

    tile_mod.TileContext._drain_and_barrier = _split


def _split_all_waits(nc):
    """Walrus here allows one sem-wait per instruction: move extra waits onto
    single-wait ENGINE_NOPs inserted just before the instruction, same engine."""
    import bass_rust
    import concourse.mybir as mybir
    nop_op = nc.isa.Opcode.NEURON_ISA_TPB_OPCODE_ENGINE_NOP
    _PE_ENGINE = mybir.EngineType.PE
    nwctr = [0]
    for f in nc.m.functions:
        for bb in f.blocks:
            out = []
            changed = False
            for inst in bb.instructions:
                si = inst.sync_info
                if si is not None and si.on_wait and len(si.on_wait) > 1:
                    changed = True
                    waits = list(si.on_wait)
                    si.on_wait = waits[-1:]
                    for w in waits[:-1]:
                        nwctr[0] += 1
                        nop = mybir.InstDrain(
                            name=f"NWD-{nwctr[0]}", ins=[], outs=[])
                        nop.engine = inst.engine
                        nop.sync_info = bass_rust.SyncInfo(
                            on_wait=[w], on_update=[])
                        out.append(nop)
                out.append(inst)
            if changed:
                bb.instructions[:] = out
    return nc


def _build_program(tb, gtiles):
    import concourse.bass as bass
    import concourse.mybir as mybir
    import concourse.tile as tile
    from concourse.tile import add_dep_helper
    from concourse.masks import make_identity

    _patch_tile_drain()
    fp = mybir.dt.float32
    AL = mybir.AluOpType
    AF = mybir.ActivationFunctionType
    tsc = NGRP * gtiles
    BIN_BIAS = 1 * 4096.0     # x' = x - 4k + 0.5 puts the slab at x-bin 1..8

    nc = bass.Bass()
    ins = {}
    for nm, cols in (("bxa", tb), ("bya", tb), ("bza", tb), ("vba", tb),
                     ("binv", tb), ("hiv", tb), ("lov", tb),
                     ("sya", tsc), ("sza", tsc), ("vla", tsc),
                     ("cyv", tsc), ("czv", tsc), ("d0x", tsc)):
        ins[nm] = nc.dram_tensor(nm, [128, cols], fp, kind="ExternalInput")
    lat_o = nc.dram_tensor("lat", [XV_PER, 128, L * T], fp, kind="ExternalOutput")
    buf_o = nc.dram_tensor("buf", [NBL * CAP, 3], fp, kind="ExternalOutput")
    msk_o = nc.dram_tensor("msk", [128, 256 * CAP], mybir.dt.uint8,
                           kind="ExternalOutput")

    with tile.TileContext(nc) as tc:
        with (tc.tile_pool(name="big", bufs=1) as big,
              tc.tile_pool(name="scr", bufs=1) as scr,
              tc.tile_pool(name="sm", bufs=2) as sm,
              tc.tile_pool(name="ps", bufs=1, space="PSUM") as ps,
              tc.tile_pool(name="psg", bufs=1, space="PSUM") as psg):

            # ---------------- constants ----------------
            ident = big.tile([128, 128], fp, tag="ident")
            make_identity(nc, ident[:])
            iota_r = big.tile([128, 256], fp, tag="iota_r")
            nc.gpsimd.iota(iota_r[:], pattern=[[1, 256]], base=0,
                           channel_multiplier=0,
                           allow_small_or_imprecise_dtypes=True)
            iota_c = big.tile([128, 1], fp, tag="iota_c")
            nc.gpsimd.iota(iota_c[:], pattern=[[0, 1]], base=0,
                           channel_multiplier=1,
                           allow_small_or_imprecise_dtypes=True)
            ltm = big.tile([128, 128], fp, tag="ltm")
            nc.gpsimd.memset(ltm[:], 1.0)
            nc.gpsimd.affine_select(out=ltm[:], in_=ltm[:], compare_op=AL.is_ge,
                                    fill=0.0, base=-1, channel_multiplier=1,
                                    pattern=[[-1, 128]])

            # lattice slab [y, x, z, t] — starts zero and also serves as the
            # zero-source for clearing buf_o
            slab = big.tile([128, XV_PER * L * T], fp, tag="slab")
            nc.vector.memset(slab[:], 0.0)
            total = NBL * CAP * 3
            step = 128 * 1024
            zero_dmas = []
            for off in range(0, total, step):
                zi = nc.gpsimd.dma_start(
                    bass.AP(buf_o, off, [[1024, 128], [1, 1024]]),
                    slab[:, :1024])
                zero_dmas.append(zi.ins)

            # ---------------- inputs ----------------
            a = {}
            for nm in ("bxa", "bya", "bza", "vba", "binv", "hiv", "lov",
                       "sya", "sza", "vla", "cyv", "czv", "d0x"):
                t_ = big.tile(list(ins[nm].shape), fp, tag=nm)
                nc.sync.dma_start(t_[:], ins[nm][:])
                a[nm] = t_

            def ts(out, in0, s1, op0, s2=None, op1=None):
                if s2 is None:
                    nc.vector.tensor_scalar(out, in0, s1, None, op0)
                else:
                    nc.vector.tensor_scalar(out, in0, s1, s2, op0, op1)

            def tt(out, i0, i1, op):
                nc.vector.tensor_tensor(out=out, in0=i0, in1=i1, op=op)

            s0 = scr.tile([128, tsc], fp, tag="s0")
            s1_ = scr.tile([128, tsc], fp, tag="s1")
            A0 = s0[:, :tb]
            A1 = s1_[:, :tb]
            binl = a["binv"]
            hiv = a["hiv"]
            lov = a["lov"]
            cyv = a["cyv"]
            czv = a["czv"]

            # ---------------- batched splat math ----------------
            pys = big.tile([128, tsc], fp, tag="pys")
            pzs = big.tile([128, tsc], fp, tag="pzs")
            wx = []
            for o in range(3):
                wxt = big.tile([128, tsc], fp, tag=f"wx{o}")
                wx.append(wxt)

            ts(pys[:], a["sya"][:], L / BOX, AL.mult, 0.5, AL.subtract)
            ts(pzs[:], a["sza"][:], L / BOX, AL.mult, 0.5, AL.subtract)
            for o in range(3):
                ts(s0[:], a["d0x"][:], float(o - 1), AL.add)
                nc.scalar.activation(s0[:], s0[:], AF.Square)
                nc.scalar.activation(wx[o][:], s0[:], AF.Exp, scale=-AEXP)
                tt(wx[o][:], wx[o][:], a["vla"][:], AL.mult)

            # ---------------- slot loop ----------------
            C = big.tile([128, 256], fp, tag="C")
            nc.gpsimd.memset(C[:], 0.0)
            slotb = big.tile([128, tb], fp, tag="slotb")
            for tau in range(tb):
                binc = binl[:, tau:tau + 1]
                hic = hiv[:, tau:tau + 1]
                loc = lov[:, tau:tau + 1]
                pTb = ps.tile([128, 128], fp, tag="pTb")
                nc.tensor.transpose(pTb[:], binc.to_broadcast([128, 128]),
                                    ident[:])
                pTh = ps.tile([128, 128], fp, tag="pTh")
                nc.tensor.transpose(pTh[:], hic.to_broadcast([128, 128]),
                                    ident[:])
                o_hiT = sm.tile([128, 128], fp, tag="o_hiT")
                nc.vector.tensor_scalar(o_hiT[:], pTh[:], iota_c[:], None,
                                        AL.is_equal)
                o_hi = sm.tile([128, 128], fp, tag="o_hi")
                nc.vector.tensor_scalar(o_hi[:], iota_r[:, :128], hic, None,
                                        AL.is_equal)
                o_lo = sm.tile([128, 256], fp, tag="o_lo")
                nc.vector.tensor_scalar(o_lo[:], iota_r[:], loc, None,
                                        AL.is_equal)
                yps = ps.tile([128, 256], fp, tag="yps")
                nc.tensor.matmul(yps[:], lhsT=o_hiT[:], rhs=C[:],
                                 start=True, stop=True)
                basec = sm.tile([128, 1], fp, tag="basec")
                tt(s0[:, :256], yps[:], o_lo[:], AL.mult)
                nc.vector.tensor_reduce(basec[:], s0[:, :256],
                                        axis=mybir.AxisListType.X, op=AL.add)
                smat = sm.tile([128, 128], fp, tag="smat")
                nc.vector.tensor_scalar(smat[:], pTb[:], binc, None,
                                        AL.is_equal)
                rc = sm.tile([128, 1], fp, tag="rc")
                tt(s1_[:, :128], smat[:], ltm[:], AL.mult)
                nc.vector.tensor_reduce(rc[:], s1_[:, :128],
                                        axis=mybir.AxisListType.X, op=AL.add)
                tt(slotb[:, tau:tau + 1], basec[:], rc[:], AL.add)
                cps = ps.tile([128, 256], fp, tag="cps")
                nc.tensor.matmul(cps[:], lhsT=o_hi[:], rhs=o_lo[:],
                                 start=True, stop=True)
                tt(C[:], C[:], cps[:], AL.add)

            # ---------------- scatter indices ----------------
            idxi = big.tile([128, tb], mybir.dt.int32, tag="idxi")
            ts(A0, binl[:], float(CAP), AL.mult)
            tt(A0, A0, slotb[:], AL.add)
            ts(A1, a["vba"][:], -1.0e9, AL.mult, 1.0e9, AL.add)
            tt(A0, A0, A1, AL.add)
            ts(A1, slotb[:], 7.5, AL.is_gt)
            ts(A1, A1, 1.0e9, AL.mult)
            tt(A0, A0, A1, AL.add)
            nc.vector.tensor_copy(idxi[:], A0)

            # ---------------- buffer mask (staged in s0 bitcast as u8) ----
            msk_t = s0[:, :512].bitcast(mybir.dt.uint8)
            nc.vector.tensor_tensor(
                out=msk_t.rearrange("p (l s) -> p l s", s=CAP),
                in0=C[:].unsqueeze(-1).to_broadcast([128, 256, CAP]),
                in1=iota_r[:, :CAP].unsqueeze(1).to_broadcast([128, 256, CAP]),
                op=AL.is_gt)
            nc.sync.dma_start(msk_o[:], msk_t)

            # ---------------- splat ----------------
            BB = 2   # tiles per build batch
            for cxl_g in range(XV_PER + 2):
                planes = [o for o in (-1, 0, 1)
                          if 0 <= cxl_g + o - 1 < XV_PER]
                for t_g in range(T):
                    g = cxl_g * T + t_g
                    if planes:
                        pls = []
                        for j in range(len(planes)):
                            plj = psg.tile([128, 128], fp, tag=f"pl{j}")
                            pls.append(plj)
                        for i0 in range(0, gtiles, BB):
                            nb = min(BB, gtiles - i0)
                            t0 = g * gtiles + i0
                            ay = sm.tile([128, 128 * BB], fp, tag="ay")
                            az = sm.tile([128, 128 * BB], fp, tag="az")
                            for dst, pcs, cvs in ((ay, pys, cyv),
                                                  (az, pzs, czv)):
                                w = nb * 128
                                gg = sm.tile([128, 128 * BB], fp, tag="gg")
                                mm_ = sm.tile([128, 128 * BB], fp, tag="mm2")
                                i3 = iota_r[:, :128].unsqueeze(1).to_broadcast(
                                    [128, nb, 128])
                                pc = pcs[:, t0:t0 + nb].unsqueeze(2)\
                                    .to_broadcast([128, nb, 128])
                                cv = cvs[:, t0:t0 + nb].unsqueeze(2)\
                                    .to_broadcast([128, nb, 128])
                                tt(gg[:, :w].rearrange("p (n z) -> p n z", n=nb),
                                   i3, pc, AL.subtract)
                                nc.scalar.activation(gg[:, :w], gg[:, :w],
                                                     AF.Square)
                                nc.scalar.activation(gg[:, :w], gg[:, :w],
                                                     AF.Exp, scale=-AEXP)
                                tt(mm_[:, :w].rearrange("p (n z) -> p n z", n=nb),
                                   i3, cv, AL.subtract)
                                nc.scalar.activation(mm_[:, :w], mm_[:, :w],
                                                     AF.Abs)
                                ts(mm_[:, :w], mm_[:, :w], 1.5, AL.is_le)
                                tt(dst[:, :w], gg[:, :w], mm_[:, :w], AL.mult)
                            for i in range(nb):
                                tsi = t0 + i
                                for j, o in enumerate(planes):
                                    azs = sm.tile([128, 128], fp, tag="azs")
                                    ts(azs[:], az[:, i * 128:(i + 1) * 128],
                                       wx[o + 1][:, tsi:tsi + 1], AL.mult)
                                    nc.tensor.matmul(
                                        pls[j][:],
                                        lhsT=ay[:, i * 128:(i + 1) * 128],
                                        rhs=azs[:],
                                        start=(i0 + i == 0),
                                        stop=(i0 + i == gtiles - 1))
                        for j, o in enumerate(planes):
                            xpl = cxl_g + o - 1
                            sl = slab[:, xpl * L * T + t_g:
                                      xpl * L * T + L * T:T]
                            tt(sl, sl, pls[j][:], AL.add)
            for xd in range(XV_PER):
                nc.sync.dma_start(lat_o[xd],
                                  slab[:, xd * L * T:(xd + 1) * L * T])

            # ---------------- buffer scatter ----------------
            rch = big.tile([128, tb, 3], fp, tag="rch")
            nc.vector.tensor_copy(rch[:, :, 0], a["bxa"][:])
            nc.vector.tensor_copy(rch[:, :, 1], a["bya"][:])
            nc.vector.tensor_copy(rch[:, :, 2], a["bza"][:])
            breg = nc.gpsimd.to_reg(NBL * CAP - 1)
            for tau in range(tb):
                sc = nc.gpsimd.indirect_dma_start(
                    out=buf_o[:],
                    out_offset=bass.IndirectOffsetOnAxis(
                        ap=idxi[:, tau:tau + 1], axis=0),
                    in_=rch[:, tau, :],
                    in_offset=None,
                    bounds_check=breg,
                    oob_is_err=False)
                for zi in zero_dmas:
                    add_dep_helper(sc.ins, zi, sync=True,
                                   reason="scatter after buf zeroing")
    _split_all_waits(nc)
    return nc


# ----------------------------------------------------------------------
def _get_program(tb, gtiles):
    key = (tb, gtiles)
    if key not in _PROG_CACHE:
        _PROG_CACHE[key] = _build_program(tb, gtiles)
    return _PROG_CACHE[key]


def kernel(points, mask, atom_types):
    from concourse.bass_utils import run_bass_kernel_spmd
    cores = _host_shard(points, mask, atom_types, TB, GTILES)
    nc = _get_program(TB, GTILES)
    res = run_bass_kernel_spmd(nc, cores, core_ids=list(range(NCORES)))
    lat = np.zeros((L, L, L, T), np.float32)
    buf = np.zeros((D ** 3, CAP, 3), np.float32)
    bmask = np.zeros((D ** 3, CAP), bool)
    for k in range(NCORES):
        r = res.results[k]
        lat[k * XV_PER:(k + 1) * XV_PER] = r["lat"].reshape(XV_PER, 128, L, T)
        buf[k * NBL:(k + 1) * NBL] = r["buf"].reshape(NBL, CAP, 3)
        bmask[k * NBL:(k + 1) * NBL] = (
            r["msk"].reshape(128 * 256, CAP)[:NBL] > 0)
    return lat, buf, bmask
